# revision 10
# baseline (speedup 1.0000x reference)
"""MoE block (grouped GEMM x2 + SwiGLU) for 8 Trainium2 NeuronCores.

Expert-parallel: 8 experts per core, tokens routed on host (inputs are
pre-sorted by expert), no on-device collectives. Per core, for each of its
8 experts e and each I-chunk i (128 wide):
  GEMM1 (PE):  psum_gu[tok=128, 256] += xT[d,tok].T @ w13[d, (gate_i|up_i)]
               accumulated over 16 d-chunks of 128
  SwiGLU:      silu(gate) (ACT) * up (DVE) -> h[tok=128, 128]
  transpose:   h -> hT[128, tok] (PE, via identity)
  GEMM2 (PE):  psum_y[tok=128, 2048] += hT.T @ w2[i-chunk, :]
               accumulated over the 11 I-chunks
Weights stream through SBUF in ~2MB/1MB contiguous DMAs (the kernel is
memory-bound: ~293MB of weights+acts per core).
"""

import sys

sys.path.insert(0, "/opt/trn_rl_repo")

import numpy as np

import concourse.bass as bass
import concourse.mybir as mybir
import concourse.tile as tile
from concourse import bacc
from concourse.bass_utils import run_bass_kernel_spmd
from concourse.masks import make_identity

E = 64
D = 2048
I = 1408
T = 8192
NCORES = 8
EPC = E // NCORES  # experts per core
P = 128

F32 = mybir.dt.float32

_prog_cache = {}


def build_nc(C=128, d=D, i_dim=I, epc=EPC, mode="f32"):
    """Build the single-core SPMD program.

    C: token capacity per expert (multiple of 128).
    mode: "f32" (exact, PE-bound) | "f32r" (TF32-like matmul, rel-err ~2e-4)
        | "bf16" (bf16-staged weights/x, rel-err ~4e-3, half the DMA bytes)
    """
    nd = d // P           # contraction chunks for GEMM1
    ni = i_dim // P       # I chunks
    tt = C // P           # token tiles per expert
    g2n = 512 if d % 512 == 0 else P  # GEMM2 output column chunk width
    ndd = d // g2n
    assert d % P == 0 and i_dim % P == 0 and C % P == 0

    mm_dt = {"f32": F32, "f32r": mybir.dt.float32r,
             "bf16": mybir.dt.bfloat16}[mode]

    nc = bacc.Bacc(None, target_bir_lowering=False)
    xt = nc.dram_tensor("xt", [epc, P, nd, C], mm_dt, kind="ExternalInput")
    w13 = nc.dram_tensor("w13", [epc, ni, P, nd, 256], mm_dt, kind="ExternalInput")
    w2 = nc.dram_tensor("w2", [epc, ni, P, d], mm_dt, kind="ExternalInput")
    y = nc.dram_tensor("y", [epc * C, d], F32, kind="ExternalOutput")

    with tile.TileContext(nc) as tc:
        with (
            tc.tile_pool(name="singles", bufs=1) as singles,
            tc.tile_pool(name="xpool", bufs=3) as xpool,
            tc.tile_pool(name="w13pool", bufs=4) as w13pool,
            tc.tile_pool(name="w2pool", bufs=3) as w2pool,
            tc.tile_pool(name="hpool", bufs=3) as hpool,
            tc.tile_pool(name="ypool", bufs=2) as ypool,
            tc.tile_pool(name="psgu", bufs=2, space="PSUM") as psgu,
            tc.tile_pool(name="pst", bufs=2, space="PSUM") as pst,
            tc.tile_pool(name="psy", bufs=1, space="PSUM") as psy,
        ):
            ident_f32 = singles.tile([P, P], F32)
            make_identity(nc, ident_f32)
            if mode == "f32":
                ident = ident_f32
            else:
                ident = singles.tile([P, P], mm_dt)
                nc.vector.tensor_copy(ident, ident_f32)

            WG = 2   # I-chunks per w13 DMA (>=1MB transfers at bf16)
            WG2 = 4  # I-chunks per w2 DMA
            for e in range(epc):
                xe = xpool.tile([P, nd, C], mm_dt, tag="xe")
                nc.sync.dma_start(out=xe, in_=xt[e])
                for t in range(tt):
                    pye = psy.tile([P, d], F32, tag="py")
                    for i0 in range(0, ni, WG):
                        gsz = min(WG, ni - i0)
                        wt = w13pool.tile([P, WG, nd, 256], mm_dt, tag="w13t")
                        nc.sync.dma_start(
                            out=wt[:, :gsz],
                            in_=w13[e, i0:i0 + gsz].rearrange("i p k c -> p i k c"),
                        )
                        if i0 % WG2 == 0:
                            g2sz = min(WG2, ni - i0)
                            w2t = w2pool.tile([P, WG2, d], mm_dt, tag="w2t")
                            nc.sync.dma_start(
                                out=w2t[:, :g2sz],
                                in_=w2[e, i0:i0 + g2sz].rearrange("i p f -> p i f"),
                            )
                        for j in range(gsz):
                            i = i0 + j
                            pgu = psgu.tile([P, 256], F32, tag="pgu")
                            for k in range(nd):
                                nc.tensor.matmul(
                                    pgu,
                                    lhsT=xe[:, k, t * P:(t + 1) * P],
                                    rhs=wt[:, j, k, :],
                                    start=(k == 0),
                                    stop=(k == nd - 1),
                                )
                            sg = hpool.tile([P, P], F32, tag="sg")
                            nc.scalar.activation(
                                sg, pgu[:, 0:P],
                                mybir.ActivationFunctionType.Sigmoid,
                            )
                            h1 = hpool.tile([P, P], F32, tag="h1")
                            nc.vector.tensor_mul(h1, sg, pgu[:, P:256])
                            h = hpool.tile([P, P], mm_dt, tag="h")
                            nc.vector.tensor_mul(h, h1, pgu[:, 0:P])
                            pt = pst.tile([P, P], mm_dt, tag="pt")
                            nc.tensor.transpose(pt, h, ident)
                            hT = hpool.tile([P, P], mm_dt, tag="hT")
                            nc.vector.tensor_copy(hT, pt)
                            for dd in range(ndd):
                                nc.tensor.matmul(
                                    pye[:, dd * g2n:(dd + 1) * g2n],
                                    lhsT=hT,
                                    rhs=w2t[:, i % WG2, dd * g2n:(dd + 1) * g2n],
                                    start=(i == 0),
                                    stop=(i == ni - 1),
                                )
                    ysb = ypool.tile([P, d], F32, tag="ysb")
                    for dd in range(ndd):
                        sl = slice(dd * g2n, (dd + 1) * g2n)
                        nc.vector.tensor_copy(ysb[:, sl], pye[:, sl])
                        nc.sync.dma_start(
                            out=y[e * C + t * P:e * C + (t + 1) * P, sl],
                            in_=ysb[:, sl],
                        )
    nc.compile()
    return nc


def _host_shard(x, counts, w13, w2, C, np_dt=np.float32):
    """Build per-core input maps (arrays staged in np_dt)."""
    offs = np.zeros(E + 1, np.int64)
    np.cumsum(counts, out=offs[1:])
    in_maps = []
    for c in range(NCORES):
        xt_c = np.zeros((EPC, P, D // P, C), np_dt)
        for le in range(EPC):
            g = c * EPC + le
            cnt = int(counts[g])
            if cnt:
                xe = x[offs[g]:offs[g] + cnt]            # [cnt, D]
                xe = xe.reshape(cnt, D // P, P)           # t, do, di
                xt_c[le, :, :, :cnt] = xe.transpose(2, 1, 0).astype(np_dt)
        wsl = w13[c * EPC:(c + 1) * EPC]                  # [EPC, D, 2I]
        # [EPC, do, di, g, i, f] -> [EPC, i, di, do, (g f)]
        w13_c = np.ascontiguousarray(
            wsl.reshape(EPC, D // P, P, 2, I // P, P)
            .transpose(0, 4, 2, 1, 3, 5)
            .reshape(EPC, I // P, P, D // P, 256)
            .astype(np_dt, copy=False)
        )
        w2_c = np.ascontiguousarray(
            w2[c * EPC:(c + 1) * EPC].reshape(EPC, I // P, P, D)
            .astype(np_dt, copy=False)
        )
        in_maps.append({"xt": xt_c, "w13": w13_c, "w2": w2_c})
    return in_maps, offs


def kernel(x, tokens_per_expert, decoding, w13, w2, _trace=False, _mode="f32r"):
    x = np.asarray(x, dtype=np.float32)
    counts = np.asarray(tokens_per_expert, dtype=np.int64)
    w13 = np.asarray(w13, dtype=np.float32)
    w2 = np.asarray(w2, dtype=np.float32)

    C = max(P, int(-(-max(counts.max(), 1) // P)) * P)

    key = (C, _mode)
    if key not in _prog_cache:
        _prog_cache[key] = build_nc(C=C, mode=key[1])
    nc = _prog_cache[key]

    if _mode == "bf16":
        import ml_dtypes
        np_dt = ml_dtypes.bfloat16
    else:
        np_dt = np.float32
    in_maps, offs = _host_shard(x, counts, w13, w2, C, np_dt=np_dt)
    res = run_bass_kernel_spmd(
        nc, in_maps, list(range(NCORES)), trace=_trace
    )

    out = np.zeros((int(counts.sum()), D), np.float32)
    for c in range(NCORES):
        yc = res.results[c]["y"]
        for le in range(EPC):
            g = c * EPC + le
            cnt = int(counts[g])
            if cnt:
                out[offs[g]:offs[g] + cnt] = yc[le * C:le * C + cnt]
    if _trace:
        return out, res
    return out


# revision 11
# speedup vs baseline: 1.0797x; 1.0797x over previous
"""MoE block (grouped GEMM x2 + SwiGLU) for 8 Trainium2 NeuronCores.

Expert-parallel: 8 experts per core, tokens routed on host (inputs are
pre-sorted by expert), no on-device collectives. Per core, for each of its
8 experts e and each I-chunk i (128 wide):
  GEMM1 (PE):  psum_gu[tok=128, 256] += xT[d,tok].T @ w13[d, (gate_i|up_i)]
               accumulated over 16 d-chunks of 128
  SwiGLU:      silu(gate) (ACT) * up (DVE) -> h[tok=128, 128]
  transpose:   h -> hT[128, tok] (PE, via identity)
  GEMM2 (PE):  psum_y[tok=128, 2048] += hT.T @ w2[i-chunk, :]
               accumulated over the 11 I-chunks
Weights stream through SBUF in ~2MB/1MB contiguous DMAs (the kernel is
memory-bound: ~293MB of weights+acts per core).
"""

import sys

sys.path.insert(0, "/opt/trn_rl_repo")

import numpy as np

import concourse.bass as bass
import concourse.mybir as mybir
import concourse.tile as tile
from concourse import bacc
from concourse.bass_utils import run_bass_kernel_spmd
from concourse.masks import make_identity

E = 64
D = 2048
I = 1408
T = 8192
NCORES = 8
EPC = E // NCORES  # experts per core
P = 128

F32 = mybir.dt.float32

_prog_cache = {}


def build_nc(C=128, d=D, i_dim=I, epc=EPC, mode="f32"):
    """Build the single-core SPMD program.

    C: token capacity per expert (multiple of 128).
    mode: "f32" (exact, PE-bound) | "f32r" (TF32-like matmul, rel-err ~2e-4)
        | "bf16" (bf16-staged weights/x, rel-err ~4e-3, half the DMA bytes)
    """
    nd = d // P           # contraction chunks for GEMM1
    ni = i_dim // P       # I chunks
    tt = C // P           # token tiles per expert
    g2n = 512 if d % 512 == 0 else P  # GEMM2 output column chunk width
    ndd = d // g2n
    assert d % P == 0 and i_dim % P == 0 and C % P == 0

    mm_dt = {"f32": F32, "f32r": mybir.dt.float32r,
             "bf16": mybir.dt.bfloat16}[mode]

    nc = bacc.Bacc(None, target_bir_lowering=False)
    xt = nc.dram_tensor("xt", [epc, P, nd, C], mm_dt, kind="ExternalInput")
    w13 = nc.dram_tensor("w13", [epc, ni, P, nd, 256], mm_dt, kind="ExternalInput")
    w2 = nc.dram_tensor("w2", [epc, ni, P, d], mm_dt, kind="ExternalInput")
    y = nc.dram_tensor("y", [epc * C, d], F32, kind="ExternalOutput")

    with tile.TileContext(nc) as tc:
        with (
            tc.tile_pool(name="singles", bufs=1) as singles,
            tc.tile_pool(name="xpool", bufs=3) as xpool,
            tc.tile_pool(name="w13pool", bufs=4) as w13pool,
            tc.tile_pool(name="w2pool", bufs=3) as w2pool,
            tc.tile_pool(name="hpool", bufs=3) as hpool,
            tc.tile_pool(name="ypool", bufs=2) as ypool,
            tc.tile_pool(name="psgu", bufs=2, space="PSUM") as psgu,
            tc.tile_pool(name="pst", bufs=2, space="PSUM") as pst,
            tc.tile_pool(name="psy", bufs=1, space="PSUM") as psy,
        ):
            ident_f32 = singles.tile([P, P], F32)
            make_identity(nc, ident_f32)
            if mode == "f32":
                ident = ident_f32
            else:
                ident = singles.tile([P, P], mm_dt)
                nc.vector.tensor_copy(ident, ident_f32)

            WG = 2   # I-chunks per w13 DMA (>=1MB transfers at bf16)
            WG2 = 4  # I-chunks per w2 DMA
            for e in range(epc):
                xe = xpool.tile([P, nd, C], mm_dt, tag="xe")
                nc.sync.dma_start(out=xe, in_=xt[e])
                for t in range(tt):
                    pye = psy.tile([P, d], F32, tag="py")
                    for i0 in range(0, ni, WG):
                        gsz = min(WG, ni - i0)
                        wt = w13pool.tile([P, WG, nd, 256], mm_dt, tag="w13t")
                        nc.sync.dma_start(
                            out=wt[:, :gsz],
                            in_=w13[e, i0:i0 + gsz].rearrange("i p k c -> p i k c"),
                        )
                        if i0 % WG2 == 0:
                            g2sz = min(WG2, ni - i0)
                            w2t = w2pool.tile([P, WG2, d], mm_dt, tag="w2t")
                            nc.sync.dma_start(
                                out=w2t[:, :g2sz],
                                in_=w2[e, i0:i0 + g2sz].rearrange("i p f -> p i f"),
                            )
                        for j in range(gsz):
                            i = i0 + j
                            pgu = psgu.tile([P, 256], F32, tag="pgu")
                            for k in range(nd):
                                nc.tensor.matmul(
                                    pgu,
                                    lhsT=xe[:, k, t * P:(t + 1) * P],
                                    rhs=wt[:, j, k, :],
                                    start=(k == 0),
                                    stop=(k == nd - 1),
                                )
                            sg = hpool.tile([P, P], F32, tag="sg")
                            nc.scalar.activation(
                                sg, pgu[:, 0:P],
                                mybir.ActivationFunctionType.Sigmoid,
                            )
                            h1 = hpool.tile([P, P], F32, tag="h1")
                            nc.vector.tensor_mul(h1, sg, pgu[:, P:256])
                            h = hpool.tile([P, P], mm_dt, tag="h")
                            nc.vector.tensor_mul(h, h1, pgu[:, 0:P])
                            pt = pst.tile([P, P], mm_dt, tag="pt")
                            nc.tensor.transpose(pt, h, ident)
                            hT = hpool.tile([P, P], mm_dt, tag="hT")
                            nc.vector.tensor_copy(hT, pt)
                            for dd in range(ndd):
                                nc.tensor.matmul(
                                    pye[:, dd * g2n:(dd + 1) * g2n],
                                    lhsT=hT,
                                    rhs=w2t[:, i % WG2, dd * g2n:(dd + 1) * g2n],
                                    start=(i == 0),
                                    stop=(i == ni - 1),
                                )
                    ysb = ypool.tile([P, d], F32, tag="ysb")
                    nc.vector.tensor_copy(ysb, pye)
                    # y goes out on the gpsimd (SWDGE) queue: it depends on
                    # this expert's full compute drain, and on the sync queue
                    # it would head-of-line-block the next expert's weight
                    # DMAs at every expert boundary.
                    nc.gpsimd.dma_start(
                        out=y[e * C + t * P:e * C + (t + 1) * P, :], in_=ysb
                    )
    nc.compile()
    return nc


def _host_shard(x, counts, w13, w2, C, np_dt=np.float32):
    """Build per-core input maps (arrays staged in np_dt)."""
    offs = np.zeros(E + 1, np.int64)
    np.cumsum(counts, out=offs[1:])
    in_maps = []
    for c in range(NCORES):
        xt_c = np.zeros((EPC, P, D // P, C), np_dt)
        for le in range(EPC):
            g = c * EPC + le
            cnt = int(counts[g])
            if cnt:
                xe = x[offs[g]:offs[g] + cnt]            # [cnt, D]
                xe = xe.reshape(cnt, D // P, P)           # t, do, di
                xt_c[le, :, :, :cnt] = xe.transpose(2, 1, 0).astype(np_dt)
        wsl = w13[c * EPC:(c + 1) * EPC]                  # [EPC, D, 2I]
        # [EPC, do, di, g, i, f] -> [EPC, i, di, do, (g f)]
        w13_c = np.ascontiguousarray(
            wsl.reshape(EPC, D // P, P, 2, I // P, P)
            .transpose(0, 4, 2, 1, 3, 5)
            .reshape(EPC, I // P, P, D // P, 256)
            .astype(np_dt, copy=False)
        )
        w2_c = np.ascontiguousarray(
            w2[c * EPC:(c + 1) * EPC].reshape(EPC, I // P, P, D)
            .astype(np_dt, copy=False)
        )
        in_maps.append({"xt": xt_c, "w13": w13_c, "w2": w2_c})
    return in_maps, offs


def kernel(x, tokens_per_expert, decoding, w13, w2, _trace=False, _mode="f32r"):
    x = np.asarray(x, dtype=np.float32)
    counts = np.asarray(tokens_per_expert, dtype=np.int64)
    w13 = np.asarray(w13, dtype=np.float32)
    w2 = np.asarray(w2, dtype=np.float32)

    C = max(P, int(-(-max(counts.max(), 1) // P)) * P)

    key = (C, _mode)
    if key not in _prog_cache:
        _prog_cache[key] = build_nc(C=C, mode=key[1])
    nc = _prog_cache[key]

    if _mode == "bf16":
        import ml_dtypes
        np_dt = ml_dtypes.bfloat16
    else:
        np_dt = np.float32
    in_maps, offs = _host_shard(x, counts, w13, w2, C, np_dt=np_dt)
    res = run_bass_kernel_spmd(
        nc, in_maps, list(range(NCORES)), trace=_trace
    )

    out = np.zeros((int(counts.sum()), D), np.float32)
    for c in range(NCORES):
        yc = res.results[c]["y"]
        for le in range(EPC):
            g = c * EPC + le
            cnt = int(counts[g])
            if cnt:
                out[offs[g]:offs[g] + cnt] = yc[le * C:le * C + cnt]
    if _trace:
        return out, res
    return out
